# revision 1
# baseline (speedup 1.0000x reference)
"""Deformable Conv2d (modulated, v2) on 8 Trainium2 NeuronCores via Bass.

v2 design, instruction-count-minimized (~275/rep vs ~2000 in v1):
- offset/mask conv -> om PSUM [96, 2048] (dy rows 0-8, dx 32-40, mk 64-72 via
  zero-padded weights; 32-aligned bases for legal DVE slicing)
- psi/anchor pipeline on [9, 2048] tap-major tiles (~25 DVE ops total)
- single-anchor gather: host-precomputed xq[r] = 4 bilinear neighbors of
  padded-image position r (1KB rows); zero padding absorbs all validity
  masking; 16 indirect DMAs per tap ([128,1] offsets)
- xbar DMA-transpose for psi/q/val tap->pixel / pixel->channel relayouts
- modulation: one stride-0-broadcast TT per tap; 4-neighbor presum via one
  strided tensor_reduce; 4 accumulating conv matmuls per tap
Sharding: data-parallel (batch=4) x (image half=2) = 8 cores.
"""
import sys

if "/opt/trn_rl_repo" not in sys.path:
    sys.path.insert(0, "/opt/trn_rl_repo")

import numpy as np
import ml_dtypes

import concourse.bass as bass
import concourse.tile as tile
import concourse.mybir as mybir
from concourse.bass_utils import run_bass_kernel_spmd

F32 = mybir.dt.float32
BF16 = mybir.dt.bfloat16
I32 = mybir.dt.int32
I16 = mybir.dt.int16
ALU = mybir.AluOpType
ACTF = mybir.ActivationFunctionType

B, C, O, H, W = 4, 128, 128, 64, 64
K2 = 9
HALVES = 2
N_CORES = B * HALVES
PIX = H * W // HALVES          # 2048 pixels per core
NPT = PIX // 128               # 16 pixel-tiles per core
HROWS = H // HALVES            # 32 image rows per core
WP = W + 2                     # padded row width (phase-1 conv)
HPAD = HROWS + 2               # 34 padded rows staged per core
PAD = 4                        # gather-table padding
PW = W + 2 * PAD               # 72
NQ = PW * PW                   # 5184 anchor rows


def _split_fat_waits(nc, proxy, max_waits=1):
    """This walrus build rejects instructions carrying more than ~1 sync wait.
    For instructions with more, offload ALL waits onto Activation-engine NoOps
    (one wait each, each incrementing a shared proxy semaphore); the original
    instruction then carries a single wait on the proxy count. This keeps the
    hot engines (DVE/PE/Pool) free of stall-chain NoOps - ACT is the cheapest
    queue here."""
    import bass_rust
    total = 0
    act = mybir.EngineType.Activation
    for f in nc.m.functions:
        for bb in f.blocks:
            newlist = []
            for ins in bb.instructions:
                si = ins.sync_info
                if si and si.on_wait and len(si.on_wait) > max_waits:
                    waits = list(si.on_wait)
                    for w in waits:
                        nop = mybir.InstNoOp(
                            name=nc.get_next_instruction_name(),
                            text_hint="fatwait_proxy",
                        )
                        nop.engine = act
                        total += 1
                        nop.sync_info = mybir.SyncInfo(
                            on_wait=[w],
                            on_update=[bass_rust.SyncUpdate(
                                sync_type="semaphore", id=proxy.num,
                                ant_name=proxy.name, update_mode="sem-inc",
                                update_value=1, update_reg=None)])
                        newlist.append(nop)
                    si.on_wait = [bass_rust.SyncWait(
                        sync_type="semaphore", id=proxy.num,
                        ant_name=proxy.name, wait_mode="sem-ge-imm",
                        wait_value=total, wait_reg=None)]
                newlist.append(ins)
            bb.instructions[:] = newlist


def _dedup_ldweights(nc):
    """The lowering emits one Ldweights per Matmult; consecutive matmuls in
    this kernel often share the same stationary weights (4 conv matmuls per
    tap). PE weights persist until the next Ldweights, so drop reloads whose
    source AP matches the previous load and that carry no sync info. The
    weight tiles here are never rewritten, so address-identity is safe."""
    pe = mybir.EngineType.PE
    for f in nc.m.functions:
        for bb in f.blocks:
            newlist = []
            last_sig = None
            for ins in bb.instructions:
                tn = type(ins).__name__
                if tn == "InstLdweights":
                    si = ins.sync_info
                    clean = not (si and (si.on_wait or si.on_update))
                    sig = repr(ins.ins[0])[:300] if ins.ins else None
                    if clean and sig is not None and sig == last_sig:
                        continue
                    last_sig = sig
                elif tn != "InstMatmult" and getattr(ins, "engine", None) == pe:
                    last_sig = None
                newlist.append(ins)
            bb.instructions[:] = newlist


def build_nc(reps=1, nq=4):
    nc = bass.Bass(num_swdge_queues=nq)
    proxy_sem = nc.alloc_semaphore("fatwait_proxy")
    tc = tile.TileContext(nc)

    x_pad = nc.dram_tensor("x_pad", [C, HPAD * WP], BF16, kind="ExternalInput")
    xq_d = nc.dram_tensor("xq", [NQ, 4 * C], BF16, kind="ExternalInput")
    wo_d = nc.dram_tensor("wo96", [C, K2 * 96], BF16, kind="ExternalInput")
    wm_d = nc.dram_tensor("w_main", [C, K2 * O], BF16, kind="ExternalInput")
    by_d = nc.dram_tensor("bY16", [K2, PIX], F32, kind="ExternalInput")
    bx_d = nc.dram_tensor("bX16", [K2, PIX], F32, kind="ExternalInput")
    mb_d = nc.dram_tensor("mb", [K2, 1], F32, kind="ExternalInput")
    out_d = nc.dram_tensor("out", [O, PIX], F32, kind="ExternalOutput")

    TT = nc.vector.tensor_tensor
    TS = nc.vector.tensor_scalar

    with tc:
        with tc.tile_pool(name="persist", bufs=1) as pp, \
             tc.tile_pool(name="work", bufs=1) as wp, \
             tc.tile_pool(name="gbuf", bufs=3) as gp, \
             tc.tile_pool(name="vbuf", bufs=2) as vp, \
             tc.tile_pool(name="psA", bufs=1, space="PSUM") as psA, \
             tc.tile_pool(name="psO", bufs=1, space="PSUM") as psO:

            # ---- persistent loads ----
            xp = pp.tile([C, HPAD * WP], BF16)
            nc.sync.dma_start(xp[:], x_pad[:])
            wo = pp.tile([C, K2 * 96], BF16)
            nc.sync.dma_start(wo[:], wo_d[:])
            wm = pp.tile([C, K2 * O], BF16)
            nc.sync.dma_start(wm[:], wm_d[:])
            bY = pp.tile([K2, PIX], F32)
            nc.sync.dma_start(bY[:], by_d[:])
            bX = pp.tile([K2, PIX], F32)
            nc.sync.dma_start(bX[:], bx_d[:])
            mb = pp.tile([K2, 1], F32)
            nc.sync.dma_start(mb[:], mb_d[:])

            q16 = pp.tile([16, PIX], I16)
            nc.vector.memset(q16[:], 0)
            psi_all = pp.tile([128, PIX], BF16)
            nc.vector.memset(psi_all[:], 0.0)
            out_sb = pp.tile([O, PIX], F32)

            def emit_om():
                # offset/mask conv -> fresh om PSUM tile (36 matmuls)
                om_ps = psA.tile([96, PIX], F32, name="om_ps")
                for g in range(4):
                    for k in range(K2):
                        ki, kj = divmod(k, 3)
                        off = (8 * g + ki) * WP
                        rhs = xp[:, off:off + 8 * WP].rearrange(
                            "c (r w) -> c r w", r=8, w=WP)[:, :, kj:kj + W]
                        nc.tensor.matmul(
                            om_ps[:, g * 512:(g + 1) * 512],
                            wo[:, k * 96:(k + 1) * 96], rhs,
                            start=(k == 0), stop=(k == K2 - 1))
                return om_ps

            # Phase 1 for rep 0; later reps' phase-1 matmuls are emitted just
            # before the previous rep's tap loop so the PE works under the
            # Pool-queue gather stream instead of extending the critical path.
            om_ps = emit_om()
            for _rep in range(reps):
                om_sb = wp.tile([96, PIX], F32, name="om_sb")
                nc.scalar.copy(om_sb[:], om_ps[:])
                dxt = wp.tile([K2, PIX], F32, name="dxt")
                nc.sync.dma_start(dxt[:], om_sb[32:32 + K2, :])
                mkt = wp.tile([K2, PIX], F32, name="mkt")
                nc.sync.dma_start(mkt[:], om_sb[64:64 + K2, :])

                # ============ Phase 2: psi + anchors (tap-major) ============
                nc.scalar.activation(mkt[:], mkt[:], ACTF.Sigmoid,
                                     bias=mb[:, 0:1])
                py = wp.tile([K2, PIX], F32, name="py")
                TT(out=py[:], in0=om_sb[0:K2, :], in1=bY[:], op=ALU.add)
                px = wp.tile([K2, PIX], F32, name="px")
                TT(out=px[:], in0=dxt[:], in1=bX[:], op=ALU.add)

                yi = wp.tile([K2, PIX], I32, name="yi")
                nc.vector.tensor_copy(yi[:], py[:])       # rounds
                y0p = wp.tile([K2, PIX], F32, name="y0p")
                nc.vector.tensor_copy(y0p[:], yi[:])
                xi = wp.tile([K2, PIX], I32, name="xi")
                nc.vector.tensor_copy(xi[:], px[:])
                x0p = wp.tile([K2, PIX], F32, name="x0p")
                nc.vector.tensor_copy(x0p[:], xi[:])

                # py/px become wy'/wx' = w - 0.5
                TT(out=py[:], in0=py[:], in1=y0p[:], op=ALU.subtract)
                TT(out=px[:], in0=px[:], in1=x0p[:], op=ALU.subtract)
                wy = wp.tile([K2, PIX], F32, name="wy")
                TS(out=wy[:], in0=py[:], scalar1=0.5, scalar2=None, op0=ALU.add)
                wx = wp.tile([K2, PIX], F32, name="wx")
                TS(out=wx[:], in0=px[:], scalar1=0.5, scalar2=None, op0=ALU.add)
                # py/px become u0/v0 = 1 - w
                TS(out=py[:], in0=py[:], scalar1=-1.0, scalar2=0.5,
                   op0=ALU.mult, op1=ALU.add)
                TS(out=px[:], in0=px[:], scalar1=-1.0, scalar2=0.5,
                   op0=ALU.mult, op1=ALU.add)

                # anchor clamps: y0 in [-4, 66] <=> y0p in [12, 82]
                TS(out=y0p[:], in0=y0p[:], scalar1=12.0, scalar2=82.0,
                   op0=ALU.max, op1=ALU.min)
                TS(out=x0p[:], in0=x0p[:], scalar1=12.0, scalar2=82.0,
                   op0=ALU.max, op1=ALU.min)

                # psi products (bf16, written to 32-aligned partition blocks)
                a0 = wp.tile([K2, PIX], F32, name="a0")
                TT(out=a0[:], in0=mkt[:], in1=py[:], op=ALU.mult)
                TT(out=mkt[:], in0=mkt[:], in1=wy[:], op=ALU.mult)  # a1
                TT(out=psi_all[0:K2, :], in0=a0[:], in1=px[:], op=ALU.mult)
                TT(out=psi_all[32:32 + K2, :], in0=a0[:], in1=wx[:], op=ALU.mult)
                TT(out=psi_all[64:64 + K2, :], in0=mkt[:], in1=px[:], op=ALU.mult)
                TT(out=psi_all[96:96 + K2, :], in0=mkt[:], in1=wx[:], op=ALU.mult)

                # anchor q = (y0c+4)*72 + x0c+4, biased coords -> -876
                TS(out=y0p[:], in0=y0p[:], scalar1=72.0, scalar2=-876.0,
                   op0=ALU.mult, op1=ALU.add)
                TT(out=y0p[:], in0=y0p[:], in1=x0p[:], op=ALU.add)
                nc.vector.tensor_copy(q16[0:K2, :], y0p[:])

                # tap-major -> pixel-major via xbar
                qT16 = wp.tile([128, NPT * 16], I16, name="qT16")
                nc.sync.dma_start(
                    qT16[:].rearrange("p (t r) -> p t r", t=NPT),
                    q16[:], transpose=True)
                qTi = wp.tile([128, NPT * 16], I32, name="qTi")
                nc.vector.tensor_copy(qTi[:], qT16[:])
                psiT = wp.tile([128, NPT * 128], BF16, name="psiT")
                nc.sync.dma_start(
                    psiT[:].rearrange("p (t r) -> p t r", t=NPT),
                    psi_all[:], transpose=True)

                # prefetch next rep's offset/mask conv under this rep's gathers
                if _rep < reps - 1:
                    om_ps = emit_om()

                # ======== Phase 3/4: gather, modulate, presum, conv ========
                p_out = psO.tile([O, PIX], F32, name="p_out")
                pT = psiT[:]
                psi_pstride = pT.ap[0][0]
                for k in range(K2):
                    gq = gp.tile([128, NPT, 4 * C], BF16, name="gq")
                    for t in range(NPT):
                        gi = nc.gpsimd.indirect_dma_start(
                            out=gq[:, t, :], out_offset=None, in_=xq_d[:],
                            in_offset=bass.IndirectOffsetOnAxis(
                                ap=qTi[:, t * 16 + k:t * 16 + k + 1], axis=0),
                        )
                        if nq > 1:
                            qn = t % nq
                            if qn:
                                try:
                                    gi.queue = f"qPoolDynamic{qn}"
                                except AttributeError:
                                    gi.inst.queue = f"qPoolDynamic{qn}"
                    # modulate in place: gq *= psi (stride-0 bcast over c)
                    in1 = bass.AP(pT.tensor, pT.offset + k,
                                  [[psi_pstride, 128], [128, NPT],
                                   [32, 4], [0, C]])
                    TT(out=gq[:].rearrange("p t (n c) -> p t n c", n=4),
                       in0=gq[:].rearrange("p t (n c) -> p t n c", n=4),
                       in1=in1, op=ALU.mult)
                    # presum 4 neighbors
                    val = vp.tile([128, NPT, C], BF16, name="val")
                    with nc.allow_low_precision("4-term bilinear presum"):
                        nc.vector.tensor_reduce(
                            val[:],
                            gq[:].rearrange("p t (n c) -> p t c n", n=4),
                            axis=mybir.AxisListType.X, op=ALU.add)
                    # pixel-major -> channel-major
                    valT = vp.tile([128, NPT, 128], BF16, name="valT")
                    nc.sync.dma_start(valT[:],
                                      val[:].rearrange("p t c -> p (t c)"),
                                      transpose=True)
                    vT = valT[:].rearrange("c t p -> c (t p)")
                    for g in range(4):
                        nc.tensor.matmul(
                            p_out[:, g * 512:(g + 1) * 512],
                            wm[:, k * O:(k + 1) * O],
                            vT[:, g * 512:(g + 1) * 512],
                            start=(k == 0), stop=(k == K2 - 1))

                # ================= Phase 5: write out =================
                nc.scalar.copy(out_sb[:], p_out[:])
                nc.sync.dma_start(out_d[:], out_sb[:])

    _split_fat_waits(nc, proxy_sem)
    nc.finalize()
    return nc


# ---------------- host-side data prep ----------------

def prep_in_maps(x, org_w, offset_w, offset_b, mask_w, mask_b):
    x = np.asarray(x, dtype=np.float32)
    org_w = np.asarray(org_w, dtype=np.float32)
    offset_w = np.asarray(offset_w, dtype=np.float32)
    offset_b = np.asarray(offset_b, dtype=np.float32)
    mask_w = np.asarray(mask_w, dtype=np.float32)
    mask_b = np.asarray(mask_b, dtype=np.float32)

    wm = org_w.reshape(O, C, K2).transpose(1, 2, 0)          # [C, K2, O]
    wm = np.ascontiguousarray(wm.reshape(C, K2 * O)).astype(ml_dtypes.bfloat16)

    # wo96: per-tap [C, 96]: cols 0-8 dy_j, 32-40 dx_j, 64-72 mk_j
    wo96 = np.zeros((C, K2, 96), np.float32)
    ow = offset_w.reshape(2 * K2, C, K2)                     # [ch, C, tap]
    mw = mask_w.reshape(K2, C, K2)
    for j in range(K2):
        wo96[:, :, j] = ow[2 * j]                            # dy_j  [C, tap]
        wo96[:, :, 32 + j] = ow[2 * j + 1]                   # dx_j
        wo96[:, :, 64 + j] = mw[j]                           # mk_j
    wo96 = np.ascontiguousarray(
        wo96.reshape(C, K2 * 96)).astype(ml_dtypes.bfloat16)

    mb = mask_b.reshape(K2, 1).astype(np.float32)

    in_maps = []
    for b in range(B):
        xb = x[b].reshape(C, H, W)
        xpadf = np.zeros((C, H + 2, WP), np.float32)
        xpadf[:, 1:H + 1, 1:W + 1] = xb

        # anchor table: padded 72x72 image, 4 neighbors per row
        xpad72 = np.zeros((PW * PW + PW + 2, C), np.float32)
        grid = xpad72[:PW * PW].reshape(PW, PW, C)
        grid[PAD:PAD + H, PAD:PAD + W] = xb.transpose(1, 2, 0)
        xq = np.concatenate(
            [xpad72[0:NQ], xpad72[1:NQ + 1],
             xpad72[PW:NQ + PW], xpad72[PW + 1:NQ + PW + 1]],
            axis=1).astype(ml_dtypes.bfloat16)               # [NQ, 4*C]

        for h in range(HALVES):
            xpad_core = np.ascontiguousarray(
                xpadf[:, HROWS * h:HROWS * h + HPAD, :].reshape(C, HPAD * WP)
            ).astype(ml_dtypes.bfloat16)
            p = np.arange(PIX)
            oy = (h * HROWS + p // W).astype(np.float32)
            ox = (p % W).astype(np.float32)
            bY = np.zeros((K2, PIX), np.float32)
            bX = np.zeros((K2, PIX), np.float32)
            for j in range(K2):
                ki, kj = divmod(j, 3)
                bY[j] = oy + ki - 1 + offset_b[2 * j] + 15.5
                bX[j] = ox + kj - 1 + offset_b[2 * j + 1] + 15.5
            in_maps.append({
                "x_pad": xpad_core, "xq": xq, "wo96": wo96, "w_main": wm,
                "bY16": bY, "bX16": bX, "mb": mb,
            })
    return in_maps


_NC_CACHE = {}


def _get_nc(reps=1):
    if reps not in _NC_CACHE:
        _NC_CACHE[reps] = build_nc(reps)
    return _NC_CACHE[reps]


def assemble(results):
    out = np.zeros((B, O, H, W), np.float32)
    for core in range(N_CORES):
        b, h = divmod(core, HALVES)
        o = np.asarray(results[core]["out"])
        out[b, :, h * HROWS:(h + 1) * HROWS, :] = o.reshape(O, HROWS, W)
    return out


def kernel(x, org_w, offset_w, offset_b, mask_w, mask_b):
    nc = _get_nc(1)
    in_maps = prep_in_maps(x, org_w, offset_w, offset_b, mask_w, mask_b)
    res = run_bass_kernel_spmd(nc, in_maps, core_ids=list(range(N_CORES)))
    return assemble(res.results)



# revision 29
# speedup vs baseline: 1.9863x; 1.9863x over previous
"""Deformable Conv2d (modulated, v4) on 8 Trainium2 NeuronCores via Bass.

This deployment charges ~40us of dispatch per *executed instruction*
regardless of op size (measured: 300 tiny DVE TTs cost ~13ms; For_i loops
crash at runtime), so v4 minimizes instruction count per rep (~180 real ops
vs ~400 in v3, ~750 in v2):
- offset/mask conv -> om PSUM [96, 2048], k-outer so Ldweights dedup keeps
  one weight load per tap (9 instead of 36)
- packed phase 2: dy@rows0-8 / dx@rows32-40 processed in single [0:41]
  partition-block ops (bY/bX stacked in one host table); mask sigmoid'd in
  place at rows 64-72; bilinear fracs (wy,wx) written straight into `ing`
- psi products computed PIXEL-major after one xbar transpose of `ing`,
  6 strided TTs directly into psi2 [128, (k*16+t)*4+n] bf16 (includes mask)
- gather indices: q i16 -> DRAM roundtrip; 9 per-tap strided DMA reads land
  them in dma_gather's "flat i at (partition i%16, col i//16)" layout;
  3 doubling DMAs replicate across the 8 Q7 core groups (the gather ucode
  reads the idx table per-core from its own 16-partition group)
- 18 dma_gather ops (1024 idx each; 2048 crashes the ucode), mlp library
  loaded via load_library + codegen_inst_isa_subclasses
- taps processed in 3 groups of 3: one modulate TT, one 4-neighbor
  tensor_reduce, one valT xbar per group instead of per tap
- fat waits split into same-engine NoOps (W-1 extra ops instead of W
  proxy NoOps + sem traffic)
Sharding: data-parallel (batch=4) x (image half=2) = 8 cores.
"""
import sys

if "/opt/trn_rl_repo" not in sys.path:
    sys.path.insert(0, "/opt/trn_rl_repo")

import numpy as np
import ml_dtypes

import concourse.bass as bass
import concourse.tile as tile
import concourse.mybir as mybir
from concourse.bass_utils import run_bass_kernel_spmd
from concourse.library_config import mlp as mlp_lib

F32 = mybir.dt.float32
BF16 = mybir.dt.bfloat16
I32 = mybir.dt.int32
I16 = mybir.dt.int16
ALU = mybir.AluOpType
ACTF = mybir.ActivationFunctionType

B, C, O, H, W = 4, 128, 128, 64, 64
K2 = 9
HALVES = 2
N_CORES = B * HALVES
PIX = H * W // HALVES          # 2048 pixels per core
NPT = PIX // 128               # 16 pixel-tiles per core
HROWS = H // HALVES            # 32 image rows per core
WP = W + 2                     # padded row width (phase-1 conv)
HPAD = HROWS + 2               # 34 padded rows staged per core
PAD = 4                        # gather-table padding
PW = W + 2 * PAD               # 72
NQ = PW * PW                   # 5184 anchor rows
KG = 3                         # taps per processing group
NGRP = K2 // KG                # 3 groups


def _split_fat_waits(nc, dummy_sem, max_waits=1):
    """This walrus build rejects instructions carrying more than ~1 sync
    wait. Offload the extra waits onto NoOps placed immediately before the
    instruction ON ITS OWN ENGINE: the engine FIFO stalls on each wait in
    turn, which is semantically the same AND-wait with W-1 extra
    instructions. Each NoOp incs a dummy sem nobody waits on (the sim
    asserts every instruction has at least one update)."""
    import bass_rust
    for f in nc.m.functions:
        for bb in f.blocks:
            newlist = []
            for ins in bb.instructions:
                si = ins.sync_info
                if si and si.on_wait and len(si.on_wait) > max_waits:
                    waits = list(si.on_wait)
                    for w in waits[:-max_waits]:
                        nop = mybir.InstNoOp(
                            name=nc.get_next_instruction_name(),
                            text_hint="fatwait_pre",
                        )
                        nop.engine = ins.engine
                        nop.sync_info = mybir.SyncInfo(
                            on_wait=[w],
                            on_update=[bass_rust.SyncUpdate(
                                sync_type="semaphore", id=dummy_sem.num,
                                ant_name=dummy_sem.name, update_mode="sem-inc",
                                update_value=1, update_reg=None)])
                        newlist.append(nop)
                    si.on_wait = waits[-max_waits:]
                newlist.append(ins)
            bb.instructions[:] = newlist


def _dedup_ldweights(nc):
    """The lowering emits one Ldweights per Matmult; consecutive matmuls
    often share the same stationary weights (k-outer loops: 4 conv matmuls
    per tap). PE weights persist until the next Ldweights, so drop reloads
    whose source AP matches the previous load and that carry no sync info."""
    pe = mybir.EngineType.PE
    for f in nc.m.functions:
        for bb in f.blocks:
            newlist = []
            last_sig = None
            for ins in bb.instructions:
                tn = type(ins).__name__
                if tn == "InstLdweights":
                    si = ins.sync_info
                    clean = not (si and (si.on_wait or si.on_update))
                    sig = repr(ins.ins[0])[:300] if ins.ins else None
                    if clean and sig is not None and sig == last_sig:
                        continue
                    last_sig = sig
                elif tn != "InstMatmult" and getattr(ins, "engine", None) == pe:
                    last_sig = None
                newlist.append(ins)
            bb.instructions[:] = newlist


def build_nc(reps=1, nq=4):
    nc = bass.Bass(num_swdge_queues=nq)
    dummy_sem = nc.alloc_semaphore("fatwait_dummy")
    tc = tile.TileContext(nc)

    x_pad = nc.dram_tensor("x_pad", [C, HPAD * WP], BF16, kind="ExternalInput")
    xq_d = nc.dram_tensor("xq", [NQ, 4 * C], BF16, kind="ExternalInput")
    wo_d = nc.dram_tensor("wo96", [C, K2 * 96], BF16, kind="ExternalInput")
    wm_d = nc.dram_tensor("w_main", [C, K2 * O], BF16, kind="ExternalInput")
    byx_d = nc.dram_tensor("bYX", [41, PIX], F32, kind="ExternalInput")
    mb_d = nc.dram_tensor("mb73", [73, 1], F32, kind="ExternalInput")
    qdram = nc.dram_tensor("qscratch", [16, PIX], I16, kind="Internal")
    out_d = nc.dram_tensor("out", [O, PIX], F32, kind="ExternalOutput")

    TT = nc.vector.tensor_tensor
    TS = nc.vector.tensor_scalar

    with tc:
        with tc.tile_pool(name="persist", bufs=1) as pp, \
             tc.tile_pool(name="work", bufs=1) as wp, \
             tc.tile_pool(name="gbuf", bufs=2) as gp, \
             tc.tile_pool(name="psA", bufs=1, space="PSUM") as psA, \
             tc.tile_pool(name="psO", bufs=1, space="PSUM") as psO:

            nc.gpsimd.load_library(mlp_lib)
            nidx_reg = nc.gpsimd.to_reg(PIX // 2)

            # ---- persistent loads ----
            xp = pp.tile([C, HPAD * WP], BF16)
            nc.sync.dma_start(xp[:], x_pad[:])
            wo = pp.tile([C, K2 * 96], BF16)
            nc.sync.dma_start(wo[:], wo_d[:])
            wm = pp.tile([C, K2 * O], BF16)
            nc.sync.dma_start(wm[:], wm_d[:])
            bYX = pp.tile([41, PIX], F32)
            nc.sync.dma_start(bYX[:], byx_d[:])
            mb = pp.tile([73, 1], F32)
            nc.sync.dma_start(mb[:], mb_d[:])

            q16 = pp.tile([16, PIX], I16)
            nc.vector.memset(q16[:], 0)
            idxall = pp.tile([128, K2 * 128], I16)
            nc.vector.memset(idxall[:], 0)
            psi2 = pp.tile([128, K2 * NPT * 4], BF16)
            ing = pp.tile([128, PIX], BF16)
            nc.vector.memset(ing[:], 0.0)
            ingT = pp.tile([128, PIX], BF16)
            out_sb = pp.tile([O, PIX], F32)

            def emit_om():
                # offset/mask conv -> om PSUM (k-outer: Ldweights dedup
                # keeps one weight load per tap)
                om_ps = psA.tile([96, PIX], F32, name="om_ps")
                if "omconv" in ABLATE:
                    nc.vector.memset(om_ps[:], 0.0)
                    return om_ps
                for k in range(K2):
                    ki, kj = divmod(k, 3)
                    for g4 in range(4):
                        off = (8 * g4 + ki) * WP
                        rhs = xp[:, off:off + 8 * WP].rearrange(
                            "c (r w) -> c r w", r=8, w=WP)[:, :, kj:kj + W]
                        nc.tensor.matmul(
                            om_ps[:, g4 * 512:(g4 + 1) * 512],
                            wo[:, k * 96:(k + 1) * 96], rhs,
                            start=(k == 0), stop=(k == K2 - 1))
                return om_ps

            om_ps = emit_om()
            for _rep in range(reps):
                # ===== Phase 2 (packed, reads om PSUM directly):
                # rows 0-8 = y-side, 32-40 = x-side
                # mask -> ing rows 64-72 (sigmoid with per-partition bias)
                nc.scalar.activation(ing[64:64 + K2, :], om_ps[64:64 + K2, :],
                                     ACTF.Sigmoid, bias=mb[64:64 + K2, 0:1])
                ppx = wp.tile([41, PIX], F32, name="ppx")
                TT(out=ppx[:], in0=om_ps[0:41, :], in1=bYX[:], op=ALU.add)
                it41 = wp.tile([41, PIX], F32, name="it41")
                nc.vector.tensor_copy(it41[:].bitcast(I32), ppx[:])  # rounds
                yx41 = wp.tile([41, PIX], F32, name="yx41")
                TS(out=yx41[:], in0=it41[:].bitcast(I32), scalar1=-0.5,
                   scalar2=None, op0=ALU.add)             # round(p)-0.5
                # bilinear fracs: wy@0-8, wx@32-40 (bf16, into ing)
                TT(out=ing[0:41, :], in0=ppx[:], in1=yx41[:], op=ALU.subtract)
                # clamp anchors: rounded in [12, 82] <=> stored [11.5, 81.5]
                TS(out=yx41[:], in0=yx41[:], scalar1=11.5, scalar2=81.5,
                   op0=ALU.max, op1=ALU.min)

                # q = 72*ya + xa - 839.5  (ya,xa = clamped round-0.5 vals)
                # ppx/it41 are dead by now: reuse their buffers (rows 0-8)
                q9 = wp.tile([41, PIX], F32, name="ppx")
                TS(out=q9[0:K2, :], in0=yx41[0:K2, :], scalar1=72.0,
                   scalar2=-839.5, op0=ALU.mult, op1=ALU.add)
                q9x = wp.tile([41, PIX], F32, name="it41")
                nc.sync.dma_start(q9x[0:K2, :], yx41[32:32 + K2, :])
                TT(out=q9[0:K2, :], in0=q9[0:K2, :], in1=q9x[0:K2, :],
                   op=ALU.add)
                nc.vector.tensor_copy(q16[0:K2, :], q9[0:K2, :])  # exact

                # ===== idx: DRAM roundtrip -> per-tap [16,128] wrap layout
                nc.sync.dma_start(qdram[:], q16[:])
                ips = idxall[:].ap[0][0]
                for k in range(K2 if "idx" not in ABLATE else 0):
                    inap = bass.AP(qdram[:].tensor, qdram[:].offset + k * PIX,
                                   [[1, 16], [16, 128]])
                    outap = bass.AP(idxall[:].tensor,
                                    idxall[:].offset + k * 128,
                                    [[ips, 16], [1, 128]])
                    nc.sync.dma_start(outap, inap)
                # replicate across the 8 Q7 core groups
                if "idx" not in ABLATE:
                    nc.sync.dma_start(idxall[16:32, :], idxall[0:16, :])
                    nc.sync.dma_start(idxall[32:64, :], idxall[0:32, :])
                    nc.sync.dma_start(idxall[64:128, :], idxall[0:64, :])

                # ===== psi: transpose ingredients to pixel-major, 6 TTs
                nc.sync.dma_start(
                    ingT[:].rearrange("p (t r) -> p t r", t=NPT),
                    ing[:], transpose=True)
                psT = ingT[:].ap[0][0]
                ps2 = psi2[:].ap[0][0]

                def ingv(base):   # ingT view [p, (k,t)] at row-block base
                    return bass.AP(ingT[:].tensor, ingT[:].offset + base,
                                   [[psT, 128], [1, K2], [128, NPT]])

                def psiv(n):      # psi2 view [p, (k,t)] at neighbor slot n
                    return bass.AP(psi2[:].tensor, psi2[:].offset + n,
                                   [[ps2, 128], [64, K2], [4, NPT]])

                TT(out=psiv(2), in0=ingv(64), in1=ingv(0), op=ALU.mult)   # a1
                TT(out=psiv(0), in0=ingv(64), in1=psiv(2), op=ALU.subtract)  # a0
                TT(out=psiv(3), in0=psiv(2), in1=ingv(32), op=ALU.mult)   # w11
                TT(out=psiv(2), in0=psiv(2), in1=psiv(3), op=ALU.subtract)  # w10
                TT(out=psiv(1), in0=psiv(0), in1=ingv(32), op=ALU.mult)   # w01
                TT(out=psiv(0), in0=psiv(0), in1=psiv(1), op=ALU.subtract)  # w00

                # prefetch next rep's offset/mask conv
                if _rep < reps - 1:
                    om_ps = emit_om()

                # ===== gather, modulate, presum, conv — 3 taps per group
                p_out = psO.tile([O, PIX], F32, name="p_out")
                for g in range(NGRP):
                    gq3 = gp.tile([128, KG * NPT, 4 * C], BF16, name="gq3")
                    for kl in range(KG):
                        k = g * KG + kl
                        for h in range(2):
                            if "gather" in ABLATE:
                                continue
                            nc.gpsimd.dma_gather(
                                gq3[:, kl * 16 + h * 8:kl * 16 + (h + 1) * 8, :],
                                xq_d[:],
                                idxall[:, k * 128 + h * 64:
                                       k * 128 + (h + 1) * 64],
                                PIX // 2, nidx_reg, 4 * C,
                                queue_num=(2 * k + h) % nq)
                    # modulate: gq3 *= psi2 (stride-0 bcast over c)
                    if "modred" in ABLATE:
                        nc.vector.memset(valT3g[g][:] if False else gq3[:, 0, 0:1], 0.0)
                    in1 = bass.AP(psi2[:].tensor, psi2[:].offset + g * KG * 64,
                                  [[ps2, 128], [4, KG * NPT], [1, 4], [0, C]])
                    TT(out=gq3[:].rearrange("p t (n c) -> p t n c", n=4),
                       in0=gq3[:].rearrange("p t (n c) -> p t n c", n=4),
                       in1=in1, op=ALU.mult)
                    # presum 4 neighbors
                    val3 = wp.tile([128, KG * NPT, C], BF16, name="val3")
                    with nc.allow_low_precision("4-term bilinear presum"):
                        nc.vector.tensor_reduce(
                            val3[:],
                            gq3[:].rearrange("p t (n c) -> p t c n", n=4),
                            axis=mybir.AxisListType.X, op=ALU.add)
                    # pixel-major -> channel-major
                    valT3 = wp.tile([128, KG * NPT, 128], BF16, name="valT3")
                    nc.sync.dma_start(
                        valT3[:],
                        val3[:].rearrange("p t c -> p (t c)"),
                        transpose=True)
                    vT = valT3[:].rearrange("c t p -> c (t p)")
                    for kl in range(KG):
                        if "mainconv" in ABLATE:
                            break
                        k = g * KG + kl
                        for g4 in range(4):
                            nc.tensor.matmul(
                                p_out[:, g4 * 512:(g4 + 1) * 512],
                                wm[:, k * O:(k + 1) * O],
                                vT[:, kl * PIX + g4 * 512:
                                   kl * PIX + (g4 + 1) * 512],
                                start=(k == 0), stop=(k == K2 - 1))

                # ===== write out
                nc.scalar.copy(out_sb[:], p_out[:])
                nc.sync.dma_start(out_d[:], out_sb[:])

    _dedup_ldweights(nc)
    _split_fat_waits(nc, dummy_sem)
    mybir.codegen_inst_isa_subclasses(nc)
    nc.finalize()
    return nc


# ---------------- host-side data prep ----------------

def prep_in_maps(x, org_w, offset_w, offset_b, mask_w, mask_b):
    x = np.asarray(x, dtype=np.float32)
    org_w = np.asarray(org_w, dtype=np.float32)
    offset_w = np.asarray(offset_w, dtype=np.float32)
    offset_b = np.asarray(offset_b, dtype=np.float32)
    mask_w = np.asarray(mask_w, dtype=np.float32)
    mask_b = np.asarray(mask_b, dtype=np.float32)

    wm = org_w.reshape(O, C, K2).transpose(1, 2, 0)          # [C, K2, O]
    wm = np.ascontiguousarray(wm.reshape(C, K2 * O)).astype(ml_dtypes.bfloat16)

    # wo96: per-tap [C, 96]: cols 0-8 dy_j, 32-40 dx_j, 64-72 mk_j
    wo96 = np.zeros((C, K2, 96), np.float32)
    ow = offset_w.reshape(2 * K2, C, K2)                     # [ch, C, tap]
    mw = mask_w.reshape(K2, C, K2)
    for j in range(K2):
        wo96[:, :, j] = ow[2 * j]                            # dy_j  [C, tap]
        wo96[:, :, 32 + j] = ow[2 * j + 1]                   # dx_j
        wo96[:, :, 64 + j] = mw[j]                           # mk_j
    wo96 = np.ascontiguousarray(
        wo96.reshape(C, K2 * 96)).astype(ml_dtypes.bfloat16)

    mb73 = np.zeros((73, 1), np.float32)
    mb73[64:64 + K2, 0] = mask_b

    in_maps = []
    for b in range(B):
        xb = x[b].reshape(C, H, W)
        xpadf = np.zeros((C, H + 2, WP), np.float32)
        xpadf[:, 1:H + 1, 1:W + 1] = xb

        # anchor table: padded 72x72 image, 4 neighbors per row
        xpad72 = np.zeros((PW * PW + PW + 2, C), np.float32)
        grid = xpad72[:PW * PW].reshape(PW, PW, C)
        grid[PAD:PAD + H, PAD:PAD + W] = xb.transpose(1, 2, 0)
        xq = np.concatenate(
            [xpad72[0:NQ], xpad72[1:NQ + 1],
             xpad72[PW:NQ + PW], xpad72[PW + 1:NQ + PW + 1]],
            axis=1).astype(ml_dtypes.bfloat16)               # [NQ, 4*C]

        for h in range(HALVES):
            xpad_core = np.ascontiguousarray(
                xpadf[:, HROWS * h:HROWS * h + HPAD, :].reshape(C, HPAD * WP)
            ).astype(ml_dtypes.bfloat16)
            p = np.arange(PIX)
            oy = (h * HROWS + p // W).astype(np.float32)
            ox = (p % W).astype(np.float32)
            bYX = np.zeros((41, PIX), np.float32)
            for j in range(K2):
                ki, kj = divmod(j, 3)
                bYX[j] = oy + ki - 1 + offset_b[2 * j] + 15.5
                bYX[32 + j] = ox + kj - 1 + offset_b[2 * j + 1] + 15.5
            in_maps.append({
                "x_pad": xpad_core, "xq": xq, "wo96": wo96, "w_main": wm,
                "bYX": bYX, "mb73": mb73,
            })
    return in_maps


_NC_CACHE = {}

# ablation switch for perf experiments: set kernel.ABLATE before build_nc
ABLATE = set()


def _get_nc(reps=1):
    if reps not in _NC_CACHE:
        _NC_CACHE[reps] = build_nc(reps)
    return _NC_CACHE[reps]


def assemble(results):
    out = np.zeros((B, O, H, W), np.float32)
    for core in range(N_CORES):
        b, h = divmod(core, HALVES)
        o = np.asarray(results[core]["out"])
        out[b, :, h * HROWS:(h + 1) * HROWS, :] = o.reshape(O, HROWS, W)
    return out


def kernel(x, org_w, offset_w, offset_b, mask_w, mask_b):
    nc = _get_nc(1)
    in_maps = prep_in_maps(x, org_w, offset_w, offset_b, mask_w, mask_b)
    res = run_bass_kernel_spmd(nc, in_maps, core_ids=list(range(N_CORES)))
    return assemble(res.results)


# revision 32
# speedup vs baseline: 3.7409x; 1.8834x over previous
"""Deformable Conv2d (modulated, v4) on 8 Trainium2 NeuronCores via Bass.

This deployment charges ~40us of dispatch per *executed instruction*
regardless of op size (measured: 300 tiny DVE TTs cost ~13ms; For_i loops
crash at runtime), so v4 minimizes instruction count per rep (~180 real ops
vs ~400 in v3, ~750 in v2):
- offset/mask conv -> om PSUM [96, 2048], k-outer so Ldweights dedup keeps
  one weight load per tap (9 instead of 36)
- packed phase 2: dy@rows0-8 / dx@rows32-40 processed in single [0:41]
  partition-block ops (bY/bX stacked in one host table); mask sigmoid'd in
  place at rows 64-72; bilinear fracs (wy,wx) written straight into `ing`
- psi products computed PIXEL-major after one xbar transpose of `ing`,
  6 strided TTs directly into psi2 [128, (k*16+t)*4+n] bf16 (includes mask)
- gather indices: q i16 -> DRAM roundtrip; 9 per-tap strided DMA reads land
  them in dma_gather's "flat i at (partition i%16, col i//16)" layout;
  3 doubling DMAs replicate across the 8 Q7 core groups (the gather ucode
  reads the idx table per-core from its own 16-partition group)
- 18 dma_gather ops (1024 idx each; 2048 crashes the ucode), mlp library
  loaded via load_library + codegen_inst_isa_subclasses
- taps processed in 3 groups of 3: one modulate TT, one 4-neighbor
  tensor_reduce, one valT xbar per group instead of per tap
- fat waits split into same-engine NoOps (W-1 extra ops instead of W
  proxy NoOps + sem traffic)
Sharding: data-parallel (batch=4) x (image half=2) = 8 cores.
"""
import sys

if "/opt/trn_rl_repo" not in sys.path:
    sys.path.insert(0, "/opt/trn_rl_repo")

import numpy as np
import ml_dtypes

import concourse.bass as bass
import concourse.tile as tile
import concourse.mybir as mybir
from concourse.bass_utils import run_bass_kernel_spmd
from concourse.library_config import mlp as mlp_lib

F32 = mybir.dt.float32
BF16 = mybir.dt.bfloat16
I32 = mybir.dt.int32
I16 = mybir.dt.int16
ALU = mybir.AluOpType
ACTF = mybir.ActivationFunctionType

B, C, O, H, W = 4, 128, 128, 64, 64
K2 = 9
HALVES = 2
N_CORES = B * HALVES
PIX = H * W // HALVES          # 2048 pixels per core
NPT = PIX // 128               # 16 pixel-tiles per core
HROWS = H // HALVES            # 32 image rows per core
WP = W + 2                     # padded row width (phase-1 conv)
HPAD = HROWS + 2               # 34 padded rows staged per core
PAD = 4                        # gather-table padding
PW = W + 2 * PAD               # 72
NQ = PW * PW                   # 5184 anchor rows
KG = 3                         # taps per processing group
NGRP = K2 // KG                # 3 groups


def _split_fat_waits(nc, dummy_sem, max_waits=1):
    """This walrus build rejects instructions carrying more than ~1 sync
    wait. Offload the extra waits onto NoOps placed immediately before the
    instruction ON ITS OWN ENGINE: the engine FIFO stalls on each wait in
    turn, which is semantically the same AND-wait with W-1 extra
    instructions. Each NoOp incs a dummy sem nobody waits on (the sim
    asserts every instruction has at least one update)."""
    import bass_rust
    for f in nc.m.functions:
        for bb in f.blocks:
            newlist = []
            for ins in bb.instructions:
                si = ins.sync_info
                if si and si.on_wait and len(si.on_wait) > max_waits:
                    waits = list(si.on_wait)
                    for w in waits[:-max_waits]:
                        nop = mybir.InstNoOp(
                            name=nc.get_next_instruction_name(),
                            text_hint="fatwait_pre",
                        )
                        nop.engine = ins.engine
                        nop.sync_info = mybir.SyncInfo(
                            on_wait=[w],
                            on_update=[bass_rust.SyncUpdate(
                                sync_type="semaphore", id=dummy_sem.num,
                                ant_name=dummy_sem.name, update_mode="sem-inc",
                                update_value=1, update_reg=None)])
                        newlist.append(nop)
                    si.on_wait = waits[-max_waits:]
                newlist.append(ins)
            bb.instructions[:] = newlist


def _dedup_ldweights(nc):
    """The lowering emits one Ldweights per Matmult; consecutive matmuls
    often share the same stationary weights (k-outer loops: 4 conv matmuls
    per tap). PE weights persist until the next Ldweights, so drop reloads
    whose source AP matches the previous load and that carry no sync info."""
    pe = mybir.EngineType.PE
    for f in nc.m.functions:
        for bb in f.blocks:
            newlist = []
            last_sig = None
            for ins in bb.instructions:
                tn = type(ins).__name__
                if tn == "InstLdweights":
                    si = ins.sync_info
                    clean = not (si and (si.on_wait or si.on_update))
                    sig = repr(ins.ins[0])[:300] if ins.ins else None
                    if clean and sig is not None and sig == last_sig:
                        continue
                    last_sig = sig
                elif tn != "InstMatmult" and getattr(ins, "engine", None) == pe:
                    last_sig = None
                newlist.append(ins)
            bb.instructions[:] = newlist


def build_nc(reps=1, nq=4):
    nc = bass.Bass(num_swdge_queues=nq)
    dummy_sem = nc.alloc_semaphore("fatwait_dummy")
    tc = tile.TileContext(nc)

    x_pad = nc.dram_tensor("x_pad", [C, HPAD * WP], BF16, kind="ExternalInput")
    xq_d = nc.dram_tensor("xq", [NQ, 4 * C], BF16, kind="ExternalInput")
    wo_d = nc.dram_tensor("wo96", [C, K2 * 96], BF16, kind="ExternalInput")
    wm_d = nc.dram_tensor("w_main", [C, K2 * O], BF16, kind="ExternalInput")
    byx_d = nc.dram_tensor("bYX", [41, PIX], F32, kind="ExternalInput")
    mb_d = nc.dram_tensor("mb73", [73, 1], F32, kind="ExternalInput")
    qdram2 = [nc.dram_tensor(f"qscratch{i}", [16, PIX], I16, kind="Internal")
              for i in range(2)]
    out_d = nc.dram_tensor("out", [O, PIX], F32, kind="ExternalOutput")

    TT = nc.vector.tensor_tensor
    TS = nc.vector.tensor_scalar

    with tc:
        with tc.tile_pool(name="persist", bufs=1) as pp, \
             tc.tile_pool(name="work", bufs=1) as wp, \
             tc.tile_pool(name="gbuf", bufs=2) as gp, \
             tc.tile_pool(name="psA", bufs=1, space="PSUM") as psA, \
             tc.tile_pool(name="psO", bufs=1, space="PSUM") as psO:

            nc.gpsimd.load_library(mlp_lib)
            nidx_reg = nc.gpsimd.to_reg(PIX // 2)

            # ---- persistent loads ----
            xp = pp.tile([C, HPAD * WP], BF16)
            nc.sync.dma_start(xp[:], x_pad[:])
            wo = pp.tile([C, K2 * 96], BF16)
            nc.sync.dma_start(wo[:], wo_d[:])
            wm = pp.tile([C, K2 * O], BF16)
            nc.sync.dma_start(wm[:], wm_d[:])
            bYX = pp.tile([41, PIX], F32)
            nc.sync.dma_start(bYX[:], byx_d[:])
            mb = pp.tile([73, 1], F32)
            nc.sync.dma_start(mb[:], mb_d[:])

            q16 = pp.tile([16, PIX], I16)
            nc.vector.memset(q16[:], 0)
            # double-buffered by rep parity: rep r+1's idx build must not
            # wait for rep r's gathers to finish reading
            idxall2 = []
            for i in range(2):
                t = pp.tile([128, K2 * 128], I16, name=f"idxall{i}")
                nc.vector.memset(t[:], 0)
                idxall2.append(t)
            psi2 = pp.tile([128, K2 * NPT * 4], BF16)
            ing = pp.tile([128, PIX], BF16)
            nc.vector.memset(ing[:], 0.0)
            ingT = pp.tile([128, PIX], BF16)
            out_sb = pp.tile([O, PIX], F32)

            def emit_om():
                # offset/mask conv -> om PSUM (k-outer: Ldweights dedup
                # keeps one weight load per tap)
                om_ps = psA.tile([96, PIX], F32, name="om_ps")
                if "omconv" in ABLATE:
                    nc.vector.memset(om_ps[:], 0.0)
                    return om_ps
                for k in range(K2):
                    ki, kj = divmod(k, 3)
                    for g4 in range(4):
                        off = (8 * g4 + ki) * WP
                        rhs = xp[:, off:off + 8 * WP].rearrange(
                            "c (r w) -> c r w", r=8, w=WP)[:, :, kj:kj + W]
                        nc.tensor.matmul(
                            om_ps[:, g4 * 512:(g4 + 1) * 512],
                            wo[:, k * 96:(k + 1) * 96], rhs,
                            start=(k == 0), stop=(k == K2 - 1))
                return om_ps

            om_ps = emit_om()
            for _rep in range(reps):
                idxall = idxall2[_rep % 2]
                qdram = qdram2[_rep % 2]
                # ===== Phase 2 (packed, reads om PSUM directly):
                # rows 0-8 = y-side, 32-40 = x-side
                # mask -> ing rows 64-72 (sigmoid with per-partition bias)
                nc.scalar.activation(ing[64:64 + K2, :], om_ps[64:64 + K2, :],
                                     ACTF.Sigmoid, bias=mb[64:64 + K2, 0:1])
                ppx = wp.tile([41, PIX], F32, name="ppx")
                TT(out=ppx[:], in0=om_ps[0:41, :], in1=bYX[:], op=ALU.add)
                it41 = wp.tile([41, PIX], F32, name="it41")
                nc.vector.tensor_copy(it41[:].bitcast(I32), ppx[:])  # rounds
                yx41 = wp.tile([41, PIX], F32, name="yx41")
                TS(out=yx41[:], in0=it41[:].bitcast(I32), scalar1=-0.5,
                   scalar2=None, op0=ALU.add)             # round(p)-0.5
                # bilinear fracs: wy@0-8, wx@32-40 (bf16, into ing)
                TT(out=ing[0:41, :], in0=ppx[:], in1=yx41[:], op=ALU.subtract)
                # clamp anchors: rounded in [12, 82] <=> stored [11.5, 81.5]
                TS(out=yx41[:], in0=yx41[:], scalar1=11.5, scalar2=81.5,
                   op0=ALU.max, op1=ALU.min)

                # q = 72*ya + xa - 839.5  (ya,xa = clamped round-0.5 vals)
                # ppx/it41 are dead by now: reuse their buffers (rows 0-8)
                q9 = wp.tile([41, PIX], F32, name="ppx")
                TS(out=q9[0:K2, :], in0=yx41[0:K2, :], scalar1=72.0,
                   scalar2=-839.5, op0=ALU.mult, op1=ALU.add)
                q9x = wp.tile([41, PIX], F32, name="it41")
                nc.sync.dma_start(q9x[0:K2, :], yx41[32:32 + K2, :])
                TT(out=q9[0:K2, :], in0=q9[0:K2, :], in1=q9x[0:K2, :],
                   op=ALU.add)
                nc.vector.tensor_copy(q16[0:K2, :], q9[0:K2, :])  # exact

                # ===== idx: DRAM roundtrip -> per-tap [16,128] wrap layout
                nc.sync.dma_start(qdram[:], q16[:])
                ips = idxall[:].ap[0][0]
                for k in range(K2 if "idx" not in ABLATE else 0):
                    inap = bass.AP(qdram[:].tensor, qdram[:].offset + k * PIX,
                                   [[1, 16], [16, 128]])
                    outap = bass.AP(idxall[:].tensor,
                                    idxall[:].offset + k * 128,
                                    [[ips, 16], [1, 128]])
                    nc.sync.dma_start(outap, inap)
                # replicate across the 8 Q7 core groups
                if "idx" not in ABLATE:
                    nc.sync.dma_start(idxall[16:32, :], idxall[0:16, :])
                    nc.sync.dma_start(idxall[32:64, :], idxall[0:32, :])
                    nc.sync.dma_start(idxall[64:128, :], idxall[0:64, :])

                # ===== psi: transpose ingredients to pixel-major, 6 TTs
                nc.sync.dma_start(
                    ingT[:].rearrange("p (t r) -> p t r", t=NPT),
                    ing[:], transpose=True)
                psT = ingT[:].ap[0][0]
                ps2 = psi2[:].ap[0][0]

                def ingv(base):   # ingT view [p, (k,t)] at row-block base
                    return bass.AP(ingT[:].tensor, ingT[:].offset + base,
                                   [[psT, 128], [1, K2], [128, NPT]])

                def psiv(n):      # psi2 view [p, (k,t)] at neighbor slot n
                    return bass.AP(psi2[:].tensor, psi2[:].offset + n,
                                   [[ps2, 128], [64, K2], [4, NPT]])

                TT(out=psiv(2), in0=ingv(64), in1=ingv(0), op=ALU.mult)   # a1
                TT(out=psiv(0), in0=ingv(64), in1=psiv(2), op=ALU.subtract)  # a0
                TT(out=psiv(3), in0=psiv(2), in1=ingv(32), op=ALU.mult)   # w11
                TT(out=psiv(2), in0=psiv(2), in1=psiv(3), op=ALU.subtract)  # w10
                TT(out=psiv(1), in0=psiv(0), in1=ingv(32), op=ALU.mult)   # w01
                TT(out=psiv(0), in0=psiv(0), in1=psiv(1), op=ALU.subtract)  # w00

                # prefetch next rep's offset/mask conv
                if _rep < reps - 1:
                    om_ps = emit_om()

                # ===== gather, modulate, presum, conv — 3 taps per group
                p_out = psO.tile([O, PIX], F32, name="p_out")
                for g in range(NGRP):
                    gq3 = gp.tile([128, KG * NPT, 4 * C], BF16, name="gq3")
                    for kl in range(KG):
                        k = g * KG + kl
                        for h in range(2):
                            if "gather" in ABLATE:
                                continue
                            nc.gpsimd.dma_gather(
                                gq3[:, kl * 16 + h * 8:kl * 16 + (h + 1) * 8, :],
                                xq_d[:],
                                idxall[:, k * 128 + h * 64:
                                       k * 128 + (h + 1) * 64],
                                PIX // 2, nidx_reg, 4 * C,
                                queue_num=(2 * k + h) % nq)
                    # modulate: gq3 *= psi2 (stride-0 bcast over c)
                    if "modred" in ABLATE:
                        nc.vector.memset(valT3g[g][:] if False else gq3[:, 0, 0:1], 0.0)
                    in1 = bass.AP(psi2[:].tensor, psi2[:].offset + g * KG * 64,
                                  [[ps2, 128], [4, KG * NPT], [1, 4], [0, C]])
                    TT(out=gq3[:].rearrange("p t (n c) -> p t n c", n=4),
                       in0=gq3[:].rearrange("p t (n c) -> p t n c", n=4),
                       in1=in1, op=ALU.mult)
                    # presum 4 neighbors
                    val3 = wp.tile([128, KG * NPT, C], BF16, name="val3")
                    with nc.allow_low_precision("4-term bilinear presum"):
                        nc.vector.tensor_reduce(
                            val3[:],
                            gq3[:].rearrange("p t (n c) -> p t c n", n=4),
                            axis=mybir.AxisListType.X, op=ALU.add)
                    # pixel-major -> channel-major
                    valT3 = wp.tile([128, KG * NPT, 128], BF16, name="valT3")
                    nc.sync.dma_start(
                        valT3[:],
                        val3[:].rearrange("p t c -> p (t c)"),
                        transpose=True)
                    vT = valT3[:].rearrange("c t p -> c (t p)")
                    for kl in range(KG):
                        if "mainconv" in ABLATE:
                            break
                        k = g * KG + kl
                        for g4 in range(4):
                            nc.tensor.matmul(
                                p_out[:, g4 * 512:(g4 + 1) * 512],
                                wm[:, k * O:(k + 1) * O],
                                vT[:, kl * PIX + g4 * 512:
                                   kl * PIX + (g4 + 1) * 512],
                                start=(k == 0), stop=(k == K2 - 1))

                # ===== write out
                nc.scalar.copy(out_sb[:], p_out[:])
                nc.sync.dma_start(out_d[:], out_sb[:])

    _dedup_ldweights(nc)
    _split_fat_waits(nc, dummy_sem)
    mybir.codegen_inst_isa_subclasses(nc)
    nc.finalize()
    return nc


# ---------------- host-side data prep ----------------

def prep_in_maps(x, org_w, offset_w, offset_b, mask_w, mask_b):
    x = np.asarray(x, dtype=np.float32)
    org_w = np.asarray(org_w, dtype=np.float32)
    offset_w = np.asarray(offset_w, dtype=np.float32)
    offset_b = np.asarray(offset_b, dtype=np.float32)
    mask_w = np.asarray(mask_w, dtype=np.float32)
    mask_b = np.asarray(mask_b, dtype=np.float32)

    wm = org_w.reshape(O, C, K2).transpose(1, 2, 0)          # [C, K2, O]
    wm = np.ascontiguousarray(wm.reshape(C, K2 * O)).astype(ml_dtypes.bfloat16)

    # wo96: per-tap [C, 96]: cols 0-8 dy_j, 32-40 dx_j, 64-72 mk_j
    wo96 = np.zeros((C, K2, 96), np.float32)
    ow = offset_w.reshape(2 * K2, C, K2)                     # [ch, C, tap]
    mw = mask_w.reshape(K2, C, K2)
    for j in range(K2):
        wo96[:, :, j] = ow[2 * j]                            # dy_j  [C, tap]
        wo96[:, :, 32 + j] = ow[2 * j + 1]                   # dx_j
        wo96[:, :, 64 + j] = mw[j]                           # mk_j
    wo96 = np.ascontiguousarray(
        wo96.reshape(C, K2 * 96)).astype(ml_dtypes.bfloat16)

    mb73 = np.zeros((73, 1), np.float32)
    mb73[64:64 + K2, 0] = mask_b

    in_maps = []
    for b in range(B):
        xb = x[b].reshape(C, H, W)
        xpadf = np.zeros((C, H + 2, WP), np.float32)
        xpadf[:, 1:H + 1, 1:W + 1] = xb

        # anchor table: padded 72x72 image, 4 neighbors per row
        xpad72 = np.zeros((PW * PW + PW + 2, C), np.float32)
        grid = xpad72[:PW * PW].reshape(PW, PW, C)
        grid[PAD:PAD + H, PAD:PAD + W] = xb.transpose(1, 2, 0)
        xq = np.concatenate(
            [xpad72[0:NQ], xpad72[1:NQ + 1],
             xpad72[PW:NQ + PW], xpad72[PW + 1:NQ + PW + 1]],
            axis=1).astype(ml_dtypes.bfloat16)               # [NQ, 4*C]

        for h in range(HALVES):
            xpad_core = np.ascontiguousarray(
                xpadf[:, HROWS * h:HROWS * h + HPAD, :].reshape(C, HPAD * WP)
            ).astype(ml_dtypes.bfloat16)
            p = np.arange(PIX)
            oy = (h * HROWS + p // W).astype(np.float32)
            ox = (p % W).astype(np.float32)
            bYX = np.zeros((41, PIX), np.float32)
            for j in range(K2):
                ki, kj = divmod(j, 3)
                bYX[j] = oy + ki - 1 + offset_b[2 * j] + 15.5
                bYX[32 + j] = ox + kj - 1 + offset_b[2 * j + 1] + 15.5
            in_maps.append({
                "x_pad": xpad_core, "xq": xq, "wo96": wo96, "w_main": wm,
                "bYX": bYX, "mb73": mb73,
            })
    return in_maps


_NC_CACHE = {}

# ablation switch for perf experiments: set kernel.ABLATE before build_nc
ABLATE = set()


def _get_nc(reps=1):
    if reps not in _NC_CACHE:
        _NC_CACHE[reps] = build_nc(reps)
    return _NC_CACHE[reps]


def assemble(results):
    out = np.zeros((B, O, H, W), np.float32)
    for core in range(N_CORES):
        b, h = divmod(core, HALVES)
        o = np.asarray(results[core]["out"])
        out[b, :, h * HROWS:(h + 1) * HROWS, :] = o.reshape(O, HROWS, W)
    return out


def kernel(x, org_w, offset_w, offset_b, mask_w, mask_b):
    nc = _get_nc(1)
    in_maps = prep_in_maps(x, org_w, offset_w, offset_b, mask_w, mask_b)
    res = run_bass_kernel_spmd(nc, in_maps, core_ids=list(range(N_CORES)))
    return assemble(res.results)


# revision 38
# speedup vs baseline: 4.2275x; 1.1301x over previous
"""Deformable Conv2d (modulated, v4) on 8 Trainium2 NeuronCores via Bass.

This deployment charges ~40us of dispatch per *executed instruction*
regardless of op size (measured: 300 tiny DVE TTs cost ~13ms; For_i loops
crash at runtime), so v4 minimizes instruction count per rep (~180 real ops
vs ~400 in v3, ~750 in v2):
- offset/mask conv -> om PSUM [96, 2048], k-outer so Ldweights dedup keeps
  one weight load per tap (9 instead of 36)
- packed phase 2: dy@rows0-8 / dx@rows32-40 processed in single [0:41]
  partition-block ops (bY/bX stacked in one host table); mask sigmoid'd in
  place at rows 64-72; bilinear fracs (wy,wx) written straight into `ing`
- psi products computed PIXEL-major after one xbar transpose of `ing`,
  6 strided TTs directly into psi2 [128, (k*16+t)*4+n] bf16 (includes mask)
- gather indices: q i16 -> DRAM roundtrip; 9 per-tap strided DMA reads land
  them in dma_gather's "flat i at (partition i%16, col i//16)" layout;
  3 doubling DMAs replicate across the 8 Q7 core groups (the gather ucode
  reads the idx table per-core from its own 16-partition group)
- 18 dma_gather ops (1024 idx each; 2048 crashes the ucode), mlp library
  loaded via load_library + codegen_inst_isa_subclasses
- taps processed in 3 groups of 3: one modulate TT, one 4-neighbor
  tensor_reduce, one valT xbar per group instead of per tap
- fat waits split into same-engine NoOps (W-1 extra ops instead of W
  proxy NoOps + sem traffic)
Sharding: data-parallel (batch=4) x (image half=2) = 8 cores.
"""
import sys

if "/opt/trn_rl_repo" not in sys.path:
    sys.path.insert(0, "/opt/trn_rl_repo")

import numpy as np
import ml_dtypes

import concourse.bass as bass
import concourse.tile as tile
import concourse.mybir as mybir
from concourse.bass_utils import run_bass_kernel_spmd
from concourse.library_config import mlp as mlp_lib

F32 = mybir.dt.float32
BF16 = mybir.dt.bfloat16
I32 = mybir.dt.int32
I16 = mybir.dt.int16
ALU = mybir.AluOpType
ACTF = mybir.ActivationFunctionType

B, C, O, H, W = 4, 128, 128, 64, 64
K2 = 9
HALVES = 2
N_CORES = B * HALVES
PIX = H * W // HALVES          # 2048 pixels per core
NPT = PIX // 128               # 16 pixel-tiles per core
HROWS = H // HALVES            # 32 image rows per core
WP = W + 2                     # padded row width (phase-1 conv)
HPAD = HROWS + 2               # 34 padded rows staged per core
PAD = 4                        # gather-table padding
PW = W + 2 * PAD               # 72
NQ = PW * PW                   # 5184 anchor rows
KG = 3                         # taps per processing group
NGRP = K2 // KG                # 3 groups


def _split_fat_waits(nc, dummy_sem, max_waits=1):
    """This walrus build rejects instructions carrying more than ~1 sync
    wait. Offload the extra waits onto NoOps placed immediately before the
    instruction ON ITS OWN ENGINE: the engine FIFO stalls on each wait in
    turn, which is semantically the same AND-wait with W-1 extra
    instructions. Each NoOp incs a dummy sem nobody waits on (the sim
    asserts every instruction has at least one update)."""
    import bass_rust
    for f in nc.m.functions:
        for bb in f.blocks:
            newlist = []
            for ins in bb.instructions:
                si = ins.sync_info
                if si and si.on_wait and len(si.on_wait) > max_waits:
                    waits = list(si.on_wait)
                    for w in waits[:-max_waits]:
                        nop = mybir.InstNoOp(
                            name=nc.get_next_instruction_name(),
                            text_hint="fatwait_pre",
                        )
                        nop.engine = ins.engine
                        nop.sync_info = mybir.SyncInfo(
                            on_wait=[w],
                            on_update=[bass_rust.SyncUpdate(
                                sync_type="semaphore", id=dummy_sem.num,
                                ant_name=dummy_sem.name, update_mode="sem-inc",
                                update_value=1, update_reg=None)])
                        newlist.append(nop)
                    si.on_wait = waits[-max_waits:]
                newlist.append(ins)
            bb.instructions[:] = newlist


def _dedup_ldweights(nc):
    """The lowering emits one Ldweights per Matmult; consecutive matmuls
    often share the same stationary weights (k-outer loops: 4 conv matmuls
    per tap). PE weights persist until the next Ldweights, so drop reloads
    whose source AP matches the previous load and that carry no sync info."""
    pe = mybir.EngineType.PE
    for f in nc.m.functions:
        for bb in f.blocks:
            newlist = []
            last_sig = None
            for ins in bb.instructions:
                tn = type(ins).__name__
                if tn == "InstLdweights":
                    si = ins.sync_info
                    clean = not (si and (si.on_wait or si.on_update))
                    sig = repr(ins.ins[0])[:300] if ins.ins else None
                    if clean and sig is not None and sig == last_sig:
                        continue
                    last_sig = sig
                elif tn != "InstMatmult" and getattr(ins, "engine", None) == pe:
                    last_sig = None
                newlist.append(ins)
            bb.instructions[:] = newlist


def build_nc(reps=1, nq=4):
    nc = bass.Bass(num_swdge_queues=nq)
    dummy_sem = nc.alloc_semaphore("fatwait_dummy")
    tc = tile.TileContext(nc)

    x_pad = nc.dram_tensor("x_pad", [C, HPAD * WP], BF16, kind="ExternalInput")
    xq_d = nc.dram_tensor("xq", [NQ, 4 * C], BF16, kind="ExternalInput")
    wo_d = nc.dram_tensor("wo96", [C, K2 * 96], BF16, kind="ExternalInput")
    wm_d = nc.dram_tensor("w_main", [C, K2 * O], BF16, kind="ExternalInput")
    byx_d = nc.dram_tensor("bYX", [41, PIX], F32, kind="ExternalInput")
    mb_d = nc.dram_tensor("mb73", [73, 1], F32, kind="ExternalInput")
    qdram2 = [nc.dram_tensor(f"qscratch{i}", [16, PIX], I16, kind="Internal")
              for i in range(2)]
    out_d = nc.dram_tensor("out", [O, PIX], F32, kind="ExternalOutput")

    TT = nc.vector.tensor_tensor
    TS = nc.vector.tensor_scalar

    with tc:
        with tc.tile_pool(name="persist", bufs=1) as pp, \
             tc.tile_pool(name="work", bufs=1) as wp, \
             tc.tile_pool(name="gbuf", bufs=2) as gp, \
             tc.tile_pool(name="psA", bufs=1, space="PSUM") as psA, \
             tc.tile_pool(name="psO", bufs=1, space="PSUM") as psO:

            nc.gpsimd.load_library(mlp_lib)
            nidx_reg = nc.gpsimd.to_reg(PIX // 2)

            # ---- persistent loads ----
            xp = pp.tile([C, HPAD * WP], BF16)
            nc.sync.dma_start(xp[:], x_pad[:])
            wo = pp.tile([C, K2 * 96], BF16)
            nc.sync.dma_start(wo[:], wo_d[:])
            wm = pp.tile([C, K2 * O], BF16)
            nc.sync.dma_start(wm[:], wm_d[:])
            bYX = pp.tile([41, PIX], F32)
            nc.sync.dma_start(bYX[:], byx_d[:])
            mb = pp.tile([73, 1], F32)
            nc.sync.dma_start(mb[:], mb_d[:])

            q16 = pp.tile([16, PIX], I16)
            nc.vector.memset(q16[:], 0)
            # double-buffered by rep parity: rep r+1's idx build must not
            # wait for rep r's gathers to finish reading
            idxall2 = []
            for i in range(2):
                t = pp.tile([128, K2 * 128], I16, name=f"idxall{i}")
                nc.vector.memset(t[:], 0)
                idxall2.append(t)
            psi2 = pp.tile([128, K2 * NPT * 4], BF16)
            ing = pp.tile([128, PIX], BF16)
            nc.vector.memset(ing[:], 0.0)
            ingT = pp.tile([128, PIX], BF16)
            out_sb = pp.tile([O, PIX], F32)

            def emit_om():
                # offset/mask conv -> om PSUM (k-outer: Ldweights dedup
                # keeps one weight load per tap)
                om_ps = psA.tile([96, PIX], F32, name="om_ps")
                if "omconv" in ABLATE:
                    nc.vector.memset(om_ps[:], 0.0)
                    return om_ps
                for k in range(K2):
                    ki, kj = divmod(k, 3)
                    for g4 in range(4):
                        off = (8 * g4 + ki) * WP
                        rhs = xp[:, off:off + 8 * WP].rearrange(
                            "c (r w) -> c r w", r=8, w=WP)[:, :, kj:kj + W]
                        nc.tensor.matmul(
                            om_ps[:, g4 * 512:(g4 + 1) * 512],
                            wo[:, k * 96:(k + 1) * 96], rhs,
                            start=(k == 0), stop=(k == K2 - 1))
                return om_ps

            om_ps = emit_om()
            for _rep in range(reps):
                idxall = idxall2[_rep % 2]
                qdram = qdram2[_rep % 2]
                # ===== Phase 2 (packed, reads om PSUM directly):
                # rows 0-8 = y-side, 32-40 = x-side
                # mask -> ing rows 64-72 (sigmoid with per-partition bias)
                nc.scalar.activation(ing[64:64 + K2, :], om_ps[64:64 + K2, :],
                                     ACTF.Sigmoid, bias=mb[64:64 + K2, 0:1])
                ppx = wp.tile([41, PIX], F32, name="ppx")
                TT(out=ppx[:], in0=om_ps[0:41, :], in1=bYX[:], op=ALU.add)
                it41 = wp.tile([41, PIX], F32, name="it41")
                nc.vector.tensor_copy(it41[:].bitcast(I32), ppx[:])  # rounds
                yx41 = wp.tile([41, PIX], F32, name="yx41")
                TS(out=yx41[:], in0=it41[:].bitcast(I32), scalar1=-0.5,
                   scalar2=None, op0=ALU.add)             # round(p)-0.5
                # bilinear fracs: wy@0-8, wx@32-40 (bf16, into ing)
                TT(out=ing[0:41, :], in0=ppx[:], in1=yx41[:], op=ALU.subtract)
                # clamp anchors: rounded in [12, 82] <=> stored [11.5, 81.5]
                TS(out=yx41[:], in0=yx41[:], scalar1=11.5, scalar2=81.5,
                   op0=ALU.max, op1=ALU.min)

                # q = 72*ya + xa - 839.5  (ya,xa = clamped round-0.5 vals)
                # ppx/it41 are dead by now: reuse their buffers (rows 0-8)
                q9 = wp.tile([41, PIX], F32, name="ppx")
                TS(out=q9[0:K2, :], in0=yx41[0:K2, :], scalar1=72.0,
                   scalar2=-839.5, op0=ALU.mult, op1=ALU.add)
                q9x = wp.tile([41, PIX], F32, name="it41")
                nc.sync.dma_start(q9x[0:K2, :], yx41[32:32 + K2, :])
                TT(out=q9[0:K2, :], in0=q9[0:K2, :], in1=q9x[0:K2, :],
                   op=ALU.add)
                nc.vector.tensor_copy(q16[0:K2, :], q9[0:K2, :])  # exact

                # ===== idx: DRAM roundtrip -> per-tap [16,128] wrap layout
                nc.sync.dma_start(qdram[:], q16[:])
                ips = idxall[:].ap[0][0]
                for k in range(K2 if "idx" not in ABLATE else 0):
                    inap = bass.AP(qdram[:].tensor, qdram[:].offset + k * PIX,
                                   [[1, 16], [16, 128]])
                    outap = bass.AP(idxall[:].tensor,
                                    idxall[:].offset + k * 128,
                                    [[ips, 16], [1, 128]])
                    # odd taps on the ACT HWDGE ring: halves the serial
                    # depth of the idx chain on the SP queue
                    eng = nc.sync if k % 2 == 0 else nc.scalar
                    eng.dma_start(outap, inap)
                # replicate across the 8 Q7 core groups
                if "idx" not in ABLATE:
                    nc.sync.dma_start(idxall[16:32, :], idxall[0:16, :])
                    nc.sync.dma_start(idxall[32:64, :], idxall[0:32, :])
                    nc.sync.dma_start(idxall[64:128, :], idxall[0:64, :])

                # ===== psi: transpose ingredients to pixel-major, 6 TTs
                nc.sync.dma_start(
                    ingT[:].rearrange("p (t r) -> p t r", t=NPT),
                    ing[:], transpose=True)
                psT = ingT[:].ap[0][0]
                ps2 = psi2[:].ap[0][0]

                def ingv(base):   # ingT view [p, (k,t)] at row-block base
                    return bass.AP(ingT[:].tensor, ingT[:].offset + base,
                                   [[psT, 128], [1, K2], [128, NPT]])

                def psiv(n):      # psi2 view [p, (k,t)] at neighbor slot n
                    return bass.AP(psi2[:].tensor, psi2[:].offset + n,
                                   [[ps2, 128], [64, K2], [4, NPT]])

                TT(out=psiv(2), in0=ingv(64), in1=ingv(0), op=ALU.mult)   # a1
                TT(out=psiv(0), in0=ingv(64), in1=psiv(2), op=ALU.subtract)  # a0
                TT(out=psiv(3), in0=psiv(2), in1=ingv(32), op=ALU.mult)   # w11
                TT(out=psiv(2), in0=psiv(2), in1=psiv(3), op=ALU.subtract)  # w10
                TT(out=psiv(1), in0=psiv(0), in1=ingv(32), op=ALU.mult)   # w01
                TT(out=psiv(0), in0=psiv(0), in1=psiv(1), op=ALU.subtract)  # w00

                # prefetch next rep's offset/mask conv
                if _rep < reps - 1:
                    om_ps = emit_om()

                # ===== gather, modulate, presum, conv — 3 taps per group
                p_out = psO.tile([O, PIX], F32, name="p_out")
                for g in range(NGRP):
                    gq3 = gp.tile([128, KG * NPT, 4 * C], BF16, name="gq3")
                    for kl in range(KG):
                        k = g * KG + kl
                        for h in range(2):
                            if "gather" in ABLATE:
                                continue
                            nc.gpsimd.dma_gather(
                                gq3[:, kl * 16 + h * 8:kl * 16 + (h + 1) * 8, :],
                                xq_d[:],
                                idxall[:, k * 128 + h * 64:
                                       k * 128 + (h + 1) * 64],
                                PIX // 2, nidx_reg, 4 * C,
                                queue_num=(2 * k + h) % nq)
                    # modulate: gq3 *= psi2 (stride-0 bcast over c)
                    if "modred" in ABLATE:
                        nc.vector.memset(valT3g[g][:] if False else gq3[:, 0, 0:1], 0.0)
                    in1 = bass.AP(psi2[:].tensor, psi2[:].offset + g * KG * 64,
                                  [[ps2, 128], [4, KG * NPT], [1, 4], [0, C]])
                    TT(out=gq3[:].rearrange("p t (n c) -> p t n c", n=4),
                       in0=gq3[:].rearrange("p t (n c) -> p t n c", n=4),
                       in1=in1, op=ALU.mult)
                    # presum 4 neighbors
                    val3 = wp.tile([128, KG * NPT, C], BF16, name="val3")
                    with nc.allow_low_precision("4-term bilinear presum"):
                        nc.vector.tensor_reduce(
                            val3[:],
                            gq3[:].rearrange("p t (n c) -> p t c n", n=4),
                            axis=mybir.AxisListType.X, op=ALU.add)
                    # pixel-major -> channel-major
                    valT3 = gp.tile([128, KG * NPT, 128], BF16, name="valT3")
                    nc.sync.dma_start(
                        valT3[:],
                        val3[:].rearrange("p t c -> p (t c)"),
                        transpose=True)
                    vT = valT3[:].rearrange("c t p -> c (t p)")
                    for kl in range(KG):
                        if "mainconv" in ABLATE:
                            break
                        k = g * KG + kl
                        for g4 in range(4):
                            nc.tensor.matmul(
                                p_out[:, g4 * 512:(g4 + 1) * 512],
                                wm[:, k * O:(k + 1) * O],
                                vT[:, kl * PIX + g4 * 512:
                                   kl * PIX + (g4 + 1) * 512],
                                start=(k == 0), stop=(k == K2 - 1))

                # ===== write out
                nc.scalar.copy(out_sb[:], p_out[:])
                nc.sync.dma_start(out_d[:], out_sb[:])

    _dedup_ldweights(nc)
    _split_fat_waits(nc, dummy_sem)
    mybir.codegen_inst_isa_subclasses(nc)
    nc.finalize()
    return nc


# ---------------- host-side data prep ----------------

def prep_in_maps(x, org_w, offset_w, offset_b, mask_w, mask_b):
    x = np.asarray(x, dtype=np.float32)
    org_w = np.asarray(org_w, dtype=np.float32)
    offset_w = np.asarray(offset_w, dtype=np.float32)
    offset_b = np.asarray(offset_b, dtype=np.float32)
    mask_w = np.asarray(mask_w, dtype=np.float32)
    mask_b = np.asarray(mask_b, dtype=np.float32)

    wm = org_w.reshape(O, C, K2).transpose(1, 2, 0)          # [C, K2, O]
    wm = np.ascontiguousarray(wm.reshape(C, K2 * O)).astype(ml_dtypes.bfloat16)

    # wo96: per-tap [C, 96]: cols 0-8 dy_j, 32-40 dx_j, 64-72 mk_j
    wo96 = np.zeros((C, K2, 96), np.float32)
    ow = offset_w.reshape(2 * K2, C, K2)                     # [ch, C, tap]
    mw = mask_w.reshape(K2, C, K2)
    for j in range(K2):
        wo96[:, :, j] = ow[2 * j]                            # dy_j  [C, tap]
        wo96[:, :, 32 + j] = ow[2 * j + 1]                   # dx_j
        wo96[:, :, 64 + j] = mw[j]                           # mk_j
    wo96 = np.ascontiguousarray(
        wo96.reshape(C, K2 * 96)).astype(ml_dtypes.bfloat16)

    mb73 = np.zeros((73, 1), np.float32)
    mb73[64:64 + K2, 0] = mask_b

    in_maps = []
    for b in range(B):
        xb = x[b].reshape(C, H, W)
        xpadf = np.zeros((C, H + 2, WP), np.float32)
        xpadf[:, 1:H + 1, 1:W + 1] = xb

        # anchor table: padded 72x72 image, 4 neighbors per row
        xpad72 = np.zeros((PW * PW + PW + 2, C), np.float32)
        grid = xpad72[:PW * PW].reshape(PW, PW, C)
        grid[PAD:PAD + H, PAD:PAD + W] = xb.transpose(1, 2, 0)
        xq = np.concatenate(
            [xpad72[0:NQ], xpad72[1:NQ + 1],
             xpad72[PW:NQ + PW], xpad72[PW + 1:NQ + PW + 1]],
            axis=1).astype(ml_dtypes.bfloat16)               # [NQ, 4*C]

        for h in range(HALVES):
            xpad_core = np.ascontiguousarray(
                xpadf[:, HROWS * h:HROWS * h + HPAD, :].reshape(C, HPAD * WP)
            ).astype(ml_dtypes.bfloat16)
            p = np.arange(PIX)
            oy = (h * HROWS + p // W).astype(np.float32)
            ox = (p % W).astype(np.float32)
            bYX = np.zeros((41, PIX), np.float32)
            for j in range(K2):
                ki, kj = divmod(j, 3)
                bYX[j] = oy + ki - 1 + offset_b[2 * j] + 15.5
                bYX[32 + j] = ox + kj - 1 + offset_b[2 * j + 1] + 15.5
            in_maps.append({
                "x_pad": xpad_core, "xq": xq, "wo96": wo96, "w_main": wm,
                "bYX": bYX, "mb73": mb73,
            })
    return in_maps


_NC_CACHE = {}

# ablation switch for perf experiments: set kernel.ABLATE before build_nc
ABLATE = set()


def _get_nc(reps=1):
    if reps not in _NC_CACHE:
        _NC_CACHE[reps] = build_nc(reps)
    return _NC_CACHE[reps]


def assemble(results):
    out = np.zeros((B, O, H, W), np.float32)
    for core in range(N_CORES):
        b, h = divmod(core, HALVES)
        o = np.asarray(results[core]["out"])
        out[b, :, h * HROWS:(h + 1) * HROWS, :] = o.reshape(O, HROWS, W)
    return out


def kernel(x, org_w, offset_w, offset_b, mask_w, mask_b):
    nc = _get_nc(1)
    in_maps = prep_in_maps(x, org_w, offset_w, offset_b, mask_w, mask_b)
    res = run_bass_kernel_spmd(nc, in_maps, core_ids=list(range(N_CORES)))
    return assemble(res.results)
